# revision 63
# baseline (speedup 1.0000x reference)
"""Multi-head self-attention Trainium2 kernel (8-core data parallel).

Reference computation (per batch b):
  q/k/v = einsum('sd,hda->hsa', x[b], W[:,i])       i in {0,1,2}
  scores = q @ k^T / sqrt(64); probs = softmax(scores)
  out = probs @ v; cat = concat heads [s, h*a]; z = cat @ Wz

Strategy per core (1 batch per core):
  - host pre-transposes x[b] -> xT [d, s] and flattens W head-major, all bf16
  - qT,kT computed W-stationary: qT[ha, s] tiles (2 heads per 128-partition
    tile); each pair's projection is interleaved into the PREVIOUS pair's
    exp-paced scores stream so the PE never idles waiting on ScalarE
  - v computed x-stationary in natural [t, ha] layout, stored per head as
    [v (64 cols) | ones (64 cols)]
  - scoresT[t, s] = kT.T @ qT per head (K=64); even/odd heads of a pair run in
    PE row-groups 0-1 / 2-3 concurrently (lhsT base partition 0 / 64)
  - exp on ScalarE with scale=1/8, no max subtraction (|scores/8| <~ 5.5)
  - one PV matmul per (head, s-half): lhsT=[v|ones] M=128 gives psum rows
    0:64 = v^T @ expT (unnorm.) and rows 64:128 = softmax denominator
    replicated across 64 partitions (matmul time is N cycles, M is free)
  - normalize is a pure DVE chain: copy den block, reciprocal_approx_fast,
    multiply -> catT[ha, s] bf16 (no cross-partition broadcast needed)
  - z^T accumulated per head pair: transient psum z-matmuls + DVE add into
    zt_sb [64, s]; each pair's z rides inside the next pair's scores stream
  - tail: 8 PE transposes to z [s, 64] fp32, DMA out; the first 4 transposes
    and the big pool teardowns are pulled into pair-7 compute
"""

import sys
from contextlib import ExitStack

sys.path.insert(0, "/opt/trn_rl_repo")

import numpy as np
import ml_dtypes

import concourse.bass as bass
import concourse.bacc as bacc
import concourse.tile as tile
import concourse.mybir as mybir
from concourse.bass_utils import run_bass_kernel_spmd
from concourse.masks import make_identity

F32 = mybir.dt.float32
BF16 = mybir.dt.bfloat16
BF = ml_dtypes.bfloat16

S = 1024  # sequence length
D = 1024  # model dim
H = 16    # heads
A = 64    # attention dim per head
B = 8     # batch (one per core)
NT = 8    # 128-row tiles per 1024 dim

TRACE = False
LAST_EXEC_NS = None

_PROGRAM = None


def _build_program():
    nc = bacc.Bacc("TRN2", target_bir_lowering=False, debug=False)

    xT = nc.dram_tensor("xT", [D, S], BF16, kind="ExternalInput").ap()
    wq = nc.dram_tensor("wq", [D, H * A], BF16, kind="ExternalInput").ap()
    wk = nc.dram_tensor("wk", [D, H * A], BF16, kind="ExternalInput").ap()
    wv = nc.dram_tensor("wv", [D, H * A], BF16, kind="ExternalInput").ap()
    wz = nc.dram_tensor("wz", [H * A, A], BF16, kind="ExternalInput").ap()
    out = nc.dram_tensor("out", [S, A], F32, kind="ExternalOutput").ap()

    with tile.TileContext(nc) as tc:
        with (
            tc.tile_pool(name="persist", bufs=1) as pers,
            tc.tile_pool(name="small", bufs=4) as small,
            tc.tile_pool(name="pssc", bufs=2, space="PSUM") as pssc,
            tc.tile_pool(name="pspv", bufs=2, space="PSUM") as pspv,
            tc.tile_pool(name="psqk", bufs=2, space="PSUM") as psqk,
        ):
            wz_sb = pers.tile([128, NT, A], BF16)
            # per head: [v (64 cols) | ones (64 cols)] so one PV matmul with
            # M=128 yields out^T on psum rows 0:64 and the softmax denominator
            # replicated on rows 64:128 (matmul time is N cycles, M is free)
            v_sb = pers.tile([128, NT, H, 2 * A], BF16)
            qt_sb = pers.tile([128, NT, S], BF16)
            kt_sb = pers.tile([128, NT, S], BF16)
            catt_sb = pers.tile([128, NT, S], BF16)
            ident = pers.tile([64, 64], F32)
            zt_sb = pers.tile([64, S], F32)
            out_sb = pers.tile([128, NT, A], F32)

            qkstack = ExitStack()
            wqkp = qkstack.enter_context(tc.tile_pool(name="wqkp", bufs=1))
            wstack = ExitStack()
            wvp = wstack.enter_context(tc.tile_pool(name="wvp", bufs=1))

            wv_sb = wvp.tile([128, NT, H * A], BF16)
            xt_sb = wqkp.tile([128, NT, S], BF16)
            wq_sb = wqkp.tile([128, NT, H * A], BF16)
            wk_sb = wqkp.tile([128, NT, H * A], BF16)

            # warmup data first so the PE can start ramping immediately
            warm_sb = pers.tile([128, 256], BF16)
            nc.vector.memset(warm_sb[:], 0.0)

            # input DMAs: wv + x first half (v phase) first, then the rest
            for d in range(NT):
                r = slice(d * 128, (d + 1) * 128)
                nc.sync.dma_start(out=wv_sb[:, d, :], in_=wv[r, :])
                nc.sync.dma_start(out=xt_sb[:, d, 0:512], in_=xT[r, 0:512])
            for d in range(NT):
                r = slice(d * 128, (d + 1) * 128)
                nc.sync.dma_start(out=xt_sb[:, d, 512:1024], in_=xT[r, 512:1024])
            for d in range(NT):
                r = slice(d * 128, (d + 1) * 128)
                nc.sync.dma_start(out=wq_sb[:, d, :], in_=wq[r, :])
                nc.sync.dma_start(out=wk_sb[:, d, :], in_=wk[r, :])
                nc.sync.dma_start(out=wz_sb[:, d, :], in_=wz[r, :])

            # ones blocks for the PV denominator rows; split across two idle
            # engines, needed only by the first PV (~45us in)
            nc.vector.memset(v_sb[:, 0:4, :, A : 2 * A], 1.0)
            nc.gpsimd.memset(v_sb[:, 4:8, :, A : 2 * A], 1.0)
            make_identity(nc, ident)

            # warmup burst: dense dummy matmuls at t=0 lift the PE HAM clock
            # gate to 8/8 before the DMA-paced V phase begins
            _wid = [0]

            def keep_warm(n):
                # dummy matmuls with no data deps: the scheduler slots them
                # into PE-idle stretches, keeping the HAM clock gate at 8/8
                _wid[0] += 1
                pw = pssc.tile([128, 1024], F32, tag="sc", name=f"warm_{_wid[0]}")
                for _ in range(n):
                    nc.tensor.matmul(
                        pw[:, 0:256], warm_sb[:, 0:128], warm_sb[:], start=True, stop=True
                    )

            keep_warm(20)

            def qk_group(hq, g):
                # one Q/K projection psum group: g selects (wq/wk, s-half)
                w_sb, dst = ((wq_sb, qt_sb), (wk_sb, kt_sb))[g // 2]
                sh = g % 2
                pq = psqk.tile([128, 512], F32, tag="qk", name=f"pq_{hq}_{g}")
                ssl = slice(sh * 512, (sh + 1) * 512)
                for d in range(NT):
                    nc.tensor.matmul(
                        pq[:],
                        w_sb[:, d, hq * 128 : (hq + 1) * 128],
                        xt_sb[:, d, ssl],
                        start=(d == 0),
                        stop=(d == NT - 1),
                    )
                nc.vector.tensor_copy(out=dst[:, hq, ssl], in_=pq[:])

            # ---- V (natural [t, ha] layout, x-stationary); pair 0's Q/K
            # projection interleaves into the last two V tiles ----
            for tt in range(NT):
                for nh in range(2):
                    pv = psqk.tile([128, 512], F32, tag="qk")
                    for d in range(NT):
                        nc.tensor.matmul(
                            pv[:],
                            xt_sb[:, d, tt * 128 : (tt + 1) * 128],
                            wv_sb[:, d, nh * 512 : (nh + 1) * 512],
                            start=(d == 0),
                            stop=(d == NT - 1),
                        )
                    nc.vector.tensor_copy(
                        out=v_sb[:, tt, nh * 8 : (nh + 1) * 8, 0:A],
                        in_=pv[:].rearrange("p (h a) -> p h a", h=8),
                    )
                    if tt >= 6:
                        qk_group(0, 2 * (tt - 6) + nh)
                if tt < 3:
                    keep_warm(8 - 2 * tt)
            wstack.close()  # frees wv_sb
            pstack = ExitStack()
            ppool = pstack.enter_context(tc.tile_pool(name="probs", bufs=23))

            # ---- fused per head-pair: attention with the NEXT pair's Q^T/K^T
            # projection interleaved into the exp-paced scores stream ----
            p2stack = None
            for hp in range(NT):

                def z_half(hz, sh):
                    # zt_sb[:, sh-half] += Wz[hz-chunk]^T @ catT[hz-chunk];
                    # cross-chunk accumulation on the DVE into SBUF so no
                    # PSUM bank is held across pairs
                    ssl = slice(sh * 512, (sh + 1) * 512)
                    pz = psqk.tile([64, 512], F32, tag="qk", name=f"pz_{hz}_{sh}")
                    nc.tensor.matmul(
                        pz[:],
                        wz_sb[:, hz, :],
                        catt_sb[:, hz, ssl],
                        start=True,
                        stop=True,
                    )
                    if hz == 0:
                        nc.vector.tensor_copy(out=zt_sb[:, ssl], in_=pz[:])
                    else:
                        nc.vector.tensor_add(zt_sb[:, ssl], zt_sb[:, ssl], pz[:])

                def z_pass(hz):
                    z_half(hz, 0)
                    z_half(hz, 1)

                if hp == NT - 1:
                    # x/wq/wk are dead (pair 7's projection ran during pair
                    # 6): free their 48KB now and give the last pair its own
                    # probs pool so its exps are not gated on earlier pairs
                    # releasing slots. Closing ppool here also moves its
                    # teardown semaphore traffic off the kernel tail.
                    pstack.close()
                    qkstack.close()
                    p2stack = ExitStack()
                    ppool2 = p2stack.enter_context(tc.tile_pool(name="probs2", bufs=10))
                    mypool = ppool2
                else:
                    mypool = ppool
                probs = [
                    [
                        mypool.tile(
                            [128, 2, 512], BF16, tag="probs", name=f"probs_{hp}_{tt}_{sh}"
                        )
                        for sh in range(2)
                    ]
                    for tt in range(NT)
                ]
                def scores_exp(tt, sh):
                    ssl = slice(sh * 512, (sh + 1) * 512)
                    ps = pssc.tile([128, 1024], F32, tag="sc", name=f"ps_{hp}_{tt}_{sh}")
                    for par in range(2):
                        po = par * 64
                        nc.tensor.matmul(
                            ps[:, par * 512 : (par + 1) * 512],
                            kt_sb[po : po + 64, hp, tt * 128 : (tt + 1) * 128],
                            qt_sb[po : po + 64, hp, ssl],
                            start=True,
                            stop=True,
                        )
                    nc.scalar.activation(
                        out=probs[tt][sh][:],
                        in_=ps[:].rearrange("p (a b) -> p a b", a=2),
                        func=mybir.ActivationFunctionType.Exp,
                        scale=0.125,
                    )

                def normalize(par, sh, po_ps):
                    # po_ps rows 0:64 = unnormalized out^T, rows 64:128 = den
                    # replicated across partitions -> pure DVE chain
                    po = par * 64
                    ssl = slice(sh * 512, (sh + 1) * 512)
                    den = small.tile([64, 512], F32, tag="den", name=f"den_{hp}_{par}_{sh}")
                    nc.vector.tensor_copy(out=den[:], in_=po_ps[A : 2 * A, :])
                    recip = small.tile([64, 512], F32, tag="recip", name=f"rc_{hp}_{par}_{sh}")
                    nc.vector.reciprocal_approx_fast(out=recip[:], in_=den[:])
                    nc.vector.tensor_mul(
                        catt_sb[po : po + 64, hp, ssl], po_ps[0:A, :], recip[:]
                    )

                def pv_mm(po_ps, tt, h, par, sh):
                    # rows 0:64 <- v^T @ expT; rows 64:128 <- den replicated
                    nc.tensor.matmul(
                        po_ps[:],
                        v_sb[:, tt, h, :],
                        probs[tt][sh][:, par, :],
                        start=(tt == 0),
                        stop=(tt == NT - 1),
                    )

                if hp < NT - 1:
                    # scores tiles 0..15; the tail of this stream is exp-paced
                    # (pssc slots recycle at ACT speed), so the previous
                    # pair's z accumulation and the NEXT pair's projection
                    # groups are slotted in there to keep the PE fed
                    i = 0
                    for tt in range(NT):
                        for sh in range(2):
                            scores_exp(tt, sh)
                            if 9 <= i <= 12:
                                qk_group(hp + 1, i - 9)
                            if i == 13 and hp > 0:
                                z_pass(hp - 1)
                            i += 1
                    for sh in range(2):
                        for par in range(2):
                            h = 2 * hp + par
                            po_ps = pspv.tile(
                                [128, 512], F32, tag="pv", name=f"pv_{h}_{sh}"
                            )
                            for tt in range(NT):
                                pv_mm(po_ps, tt, h, par, sh)
                            normalize(par, sh, po_ps)
                else:
                    # last pair: interleave PV with scores/exp per t-tile so
                    # the PE keeps work during the exp-paced pipeline drain;
                    # the sh0 half of z + the first four output transposes
                    # ride inside / right after the sh1 stream so only the
                    # sh1 half remains in the tail
                    trig = [nc.sync, nc.scalar, nc.gpsimd]

                    def emit_out(st):
                        # transpose z^T[:, st-tile] -> z [s, 64] and DMA out;
                        # psum from psqk (pspv still holds the live PV tiles)
                        pt = psqk.tile([128, 512], F32, tag="qk", name=f"pt_{st}")
                        nc.tensor.transpose(
                            pt[:, 0:A], zt_sb[:, st * 128 : (st + 1) * 128], ident[:]
                        )
                        nc.vector.tensor_copy(out=out_sb[:, st, :], in_=pt[:, 0:A])
                        trig[st % 3].dma_start(
                            out=out.rearrange("(st p) n -> p st n", p=128)[:, st, :],
                            in_=out_sb[:, st, :],
                        )

                    for sh in range(2):
                        pvt = [
                            pspv.tile([128, 512], F32, tag="pv", name=f"pvL_{par}_{sh}")
                            for par in range(2)
                        ]
                        for tt in range(NT):
                            scores_exp(tt, sh)
                            if sh == 0 and tt == 6:
                                z_pass(hp - 1)
                            if sh == 1 and tt == 6:
                                z_half(NT - 1, 0)
                            for par in range(2):
                                pv_mm(pvt[par], tt, 2 * hp + par, par, sh)
                        if sh == 1:
                            # zt_sb[:, 0:512] is final: move its transposes
                            # off the tail while the sh1 normalize drains
                            for st in range(4):
                                emit_out(st)
                            # hold the HAM clock gate open across the ~4us
                            # normalize-chain wait so the final z matmul and
                            # transposes run warm instead of at 1.2 GHz
                            keep_warm(8)
                        for par in range(2):
                            normalize(par, sh, pvt[par])
            # final s-half of z; probs2 teardown overlaps the tail
            z_half(NT - 1, 1)
            p2stack.close()
            for st in range(4, NT):
                emit_out(st)

    nc.compile()
    return nc


def _get_program():
    global _PROGRAM
    if _PROGRAM is None:
        _PROGRAM = _build_program()
    return _PROGRAM


def kernel(x: np.ndarray, W: np.ndarray, Wz: np.ndarray) -> np.ndarray:
    global LAST_EXEC_NS
    x = np.asarray(x, dtype=np.float32)
    W = np.asarray(W, dtype=np.float32)
    Wz = np.asarray(Wz, dtype=np.float32)
    assert x.shape == (B, S, D) and W.shape == (H, 3, D, A) and Wz.shape == (H * A, A)

    # host-side prep: flatten weights head-major [d, h*a], cast to bf16
    Wf = W.astype(BF)
    wq_h = np.ascontiguousarray(Wf[:, 0].transpose(1, 0, 2).reshape(D, H * A))
    wk_h = np.ascontiguousarray(Wf[:, 1].transpose(1, 0, 2).reshape(D, H * A))
    wv_h = np.ascontiguousarray(Wf[:, 2].transpose(1, 0, 2).reshape(D, H * A))
    wz_h = np.ascontiguousarray(Wz.astype(BF))

    in_maps = []
    for b in range(B):
        xt = np.ascontiguousarray(x[b].T.astype(BF))
        in_maps.append({"xT": xt, "wq": wq_h, "wk": wk_h, "wv": wv_h, "wz": wz_h})

    nc = _get_program()
    last_exc = None
    for attempt in range(3):
        try:
            res = run_bass_kernel_spmd(nc, in_maps, core_ids=list(range(B)), trace=TRACE)
            break
        except Exception as e:  # transient device faults (e.g. NRT unrecoverable)
            last_exc = e
            import time

            time.sleep(2.0)
    else:
        raise last_exc
    LAST_EXEC_NS = res.exec_time_ns
    return np.stack([res.results[b]["out"] for b in range(B)], axis=0)


# revision 65
# speedup vs baseline: 1.0277x; 1.0277x over previous
"""Multi-head self-attention Trainium2 kernel (8-core data parallel).

Reference computation (per batch b):
  q/k/v = einsum('sd,hda->hsa', x[b], W[:,i])       i in {0,1,2}
  scores = q @ k^T / sqrt(64); probs = softmax(scores)
  out = probs @ v; cat = concat heads [s, h*a]; z = cat @ Wz

Strategy per core (1 batch per core):
  - host pre-transposes x[b] -> xT [d, s] and flattens W head-major, all bf16
  - qT,kT computed W-stationary: qT[ha, s] tiles (2 heads per 128-partition
    tile); each pair's projection is interleaved into the PREVIOUS pair's
    exp-paced scores stream so the PE never idles waiting on ScalarE
  - v computed x-stationary in natural [t, ha] layout, stored per head as
    [v (64 cols) | ones (64 cols)]
  - scoresT[t, s] = kT.T @ qT per head (K=64); even/odd heads of a pair run in
    PE row-groups 0-1 / 2-3 concurrently (lhsT base partition 0 / 64)
  - exp on ScalarE with scale=1/8, no max subtraction (|scores/8| <~ 5.5)
  - one PV matmul per (head, s-half): lhsT=[v|ones] M=128 gives psum rows
    0:64 = v^T @ expT (unnorm.) and rows 64:128 = softmax denominator
    replicated across 64 partitions (matmul time is N cycles, M is free)
  - normalize is a pure DVE chain: copy den block, reciprocal_approx_fast,
    multiply -> catT[ha, s] bf16 (no cross-partition broadcast needed)
  - z^T accumulated per head pair: transient psum z-matmuls + DVE add into
    zt_sb [64, s]; each pair's z rides inside the next pair's scores stream
  - tail: 8 PE transposes to z [s, 64] fp32, DMA out; the first 4 transposes
    and the big pool teardowns are pulled into pair-7 compute
"""

import sys
from contextlib import ExitStack

sys.path.insert(0, "/opt/trn_rl_repo")

import numpy as np
import ml_dtypes

import concourse.bass as bass
import concourse.bacc as bacc
import concourse.tile as tile
import concourse.mybir as mybir
from concourse.bass_utils import run_bass_kernel_spmd
from concourse.masks import make_identity

F32 = mybir.dt.float32
BF16 = mybir.dt.bfloat16
BF = ml_dtypes.bfloat16

S = 1024  # sequence length
D = 1024  # model dim
H = 16    # heads
A = 64    # attention dim per head
B = 8     # batch (one per core)
NT = 8    # 128-row tiles per 1024 dim

TRACE = False
LAST_EXEC_NS = None

_PROGRAM = None


def _build_program():
    nc = bacc.Bacc("TRN2", target_bir_lowering=False, debug=False)

    xT = nc.dram_tensor("xT", [D, S], BF16, kind="ExternalInput").ap()
    wq = nc.dram_tensor("wq", [D, H * A], BF16, kind="ExternalInput").ap()
    wk = nc.dram_tensor("wk", [D, H * A], BF16, kind="ExternalInput").ap()
    wv = nc.dram_tensor("wv", [D, H * A], BF16, kind="ExternalInput").ap()
    wz = nc.dram_tensor("wz", [H * A, A], BF16, kind="ExternalInput").ap()
    out = nc.dram_tensor("out", [S, A], F32, kind="ExternalOutput").ap()

    with tile.TileContext(nc) as tc:
        with (
            tc.tile_pool(name="persist", bufs=1) as pers,
            tc.tile_pool(name="small", bufs=4) as small,
            tc.tile_pool(name="pssc", bufs=2, space="PSUM") as pssc,
            tc.tile_pool(name="pspv", bufs=2, space="PSUM") as pspv,
            tc.tile_pool(name="psqk", bufs=2, space="PSUM") as psqk,
        ):
            wz_sb = pers.tile([128, NT, A], BF16)
            # per head: [v (64 cols) | ones (64 cols)] so one PV matmul with
            # M=128 yields out^T on psum rows 0:64 and the softmax denominator
            # replicated on rows 64:128 (matmul time is N cycles, M is free)
            v_sb = pers.tile([128, NT, H, 2 * A], BF16)
            qt_sb = pers.tile([128, NT, S], BF16)
            kt_sb = pers.tile([128, NT, S], BF16)
            catt_sb = pers.tile([128, NT, S], BF16)
            ident = pers.tile([64, 64], F32)
            zt_sb = pers.tile([64, S], F32)
            out_sb = pers.tile([128, NT, A], F32)

            qkstack = ExitStack()
            wqkp = qkstack.enter_context(tc.tile_pool(name="wqkp", bufs=1))
            wstack = ExitStack()
            wvp = wstack.enter_context(tc.tile_pool(name="wvp", bufs=1))

            wv_sb = wvp.tile([128, NT, H * A], BF16)
            xt_sb = wqkp.tile([128, NT, S], BF16)
            wq_sb = wqkp.tile([128, NT, H * A], BF16)
            wk_sb = wqkp.tile([128, NT, H * A], BF16)

            # warmup data first so the PE can start ramping immediately
            warm_sb = pers.tile([128, 256], BF16)
            nc.vector.memset(warm_sb[:], 0.0)

            # input DMAs: wv + x first half (v phase) first, then the rest
            for d in range(NT):
                r = slice(d * 128, (d + 1) * 128)
                nc.sync.dma_start(out=wv_sb[:, d, :], in_=wv[r, :])
                nc.sync.dma_start(out=xt_sb[:, d, 0:512], in_=xT[r, 0:512])
            for d in range(NT):
                r = slice(d * 128, (d + 1) * 128)
                nc.sync.dma_start(out=xt_sb[:, d, 512:1024], in_=xT[r, 512:1024])
            for d in range(NT):
                r = slice(d * 128, (d + 1) * 128)
                nc.sync.dma_start(out=wq_sb[:, d, :], in_=wq[r, :])
                nc.sync.dma_start(out=wk_sb[:, d, :], in_=wk[r, :])
                nc.sync.dma_start(out=wz_sb[:, d, :], in_=wz[r, :])

            # ones blocks for the PV denominator rows; split across two idle
            # engines, needed only by the first PV (~45us in)
            nc.vector.memset(v_sb[:, 0:4, :, A : 2 * A], 1.0)
            nc.gpsimd.memset(v_sb[:, 4:8, :, A : 2 * A], 1.0)
            make_identity(nc, ident)

            # warmup burst: dense dummy matmuls at t=0 lift the PE HAM clock
            # gate to 8/8 before the DMA-paced V phase begins
            _wid = [0]

            def keep_warm(n):
                # dummy matmuls with no data deps: the scheduler slots them
                # into PE-idle stretches, keeping the HAM clock gate at 8/8
                _wid[0] += 1
                pw = pssc.tile([128, 1024], F32, tag="sc", name=f"warm_{_wid[0]}")
                for _ in range(n):
                    nc.tensor.matmul(
                        pw[:, 0:256], warm_sb[:, 0:128], warm_sb[:], start=True, stop=True
                    )

            keep_warm(20)

            def qk_group(hq, g):
                # one Q/K projection psum group: g selects (wq/wk, s-half)
                w_sb, dst = ((wq_sb, qt_sb), (wk_sb, kt_sb))[g // 2]
                sh = g % 2
                pq = psqk.tile([128, 512], F32, tag="qk", name=f"pq_{hq}_{g}")
                ssl = slice(sh * 512, (sh + 1) * 512)
                for d in range(NT):
                    nc.tensor.matmul(
                        pq[:],
                        w_sb[:, d, hq * 128 : (hq + 1) * 128],
                        xt_sb[:, d, ssl],
                        start=(d == 0),
                        stop=(d == NT - 1),
                    )
                nc.vector.tensor_copy(out=dst[:, hq, ssl], in_=pq[:])

            # ---- V (natural [t, ha] layout, x-stationary); pair 0's Q/K
            # projection interleaves into the last two V tiles ----
            for tt in range(NT):
                for nh in range(2):
                    pv = psqk.tile([128, 512], F32, tag="qk")
                    for d in range(NT):
                        nc.tensor.matmul(
                            pv[:],
                            xt_sb[:, d, tt * 128 : (tt + 1) * 128],
                            wv_sb[:, d, nh * 512 : (nh + 1) * 512],
                            start=(d == 0),
                            stop=(d == NT - 1),
                        )
                    nc.vector.tensor_copy(
                        out=v_sb[:, tt, nh * 8 : (nh + 1) * 8, 0:A],
                        in_=pv[:].rearrange("p (h a) -> p h a", h=8),
                    )
                    if tt >= 6:
                        qk_group(0, 2 * (tt - 6) + nh)
                if tt < 3:
                    keep_warm(8 - 2 * tt)
            wstack.close()  # frees wv_sb
            pstack = ExitStack()
            ppool = pstack.enter_context(tc.tile_pool(name="probs", bufs=23))

            # ---- fused per head-pair: attention with the NEXT pair's Q^T/K^T
            # projection interleaved into the exp-paced scores stream ----
            p2stack = None
            for hp in range(NT):

                def z_half(hz, sh):
                    # zt_sb[:, sh-half] += Wz[hz-chunk]^T @ catT[hz-chunk];
                    # cross-chunk accumulation on the DVE into SBUF so no
                    # PSUM bank is held across pairs
                    ssl = slice(sh * 512, (sh + 1) * 512)
                    pz = psqk.tile([64, 512], F32, tag="qk", name=f"pz_{hz}_{sh}")
                    nc.tensor.matmul(
                        pz[:],
                        wz_sb[:, hz, :],
                        catt_sb[:, hz, ssl],
                        start=True,
                        stop=True,
                    )
                    if hz == 0:
                        nc.vector.tensor_copy(out=zt_sb[:, ssl], in_=pz[:])
                    else:
                        nc.vector.tensor_add(zt_sb[:, ssl], zt_sb[:, ssl], pz[:])

                def z_pass(hz):
                    z_half(hz, 0)
                    z_half(hz, 1)

                if hp == NT - 1:
                    # x/wq/wk are dead (pair 7's projection ran during pair
                    # 6): free their 48KB now and give the last pair its own
                    # probs pool so its exps are not gated on earlier pairs
                    # releasing slots. Closing ppool here also moves its
                    # teardown semaphore traffic off the kernel tail.
                    pstack.close()
                    qkstack.close()
                    p2stack = ExitStack()
                    ppool2 = p2stack.enter_context(tc.tile_pool(name="probs2", bufs=10))
                    mypool = ppool2
                else:
                    mypool = ppool
                # allocation order must match consumption order for the pool
                # ring: sh-major for the reordered pairs, tt-major for the
                # last pair's interleaved drain
                probs = [[None] * 2 for _ in range(NT)]
                if hp < NT - 1:
                    it = [(tt, sh) for sh in range(2) for tt in range(NT)]
                else:
                    it = [(tt, sh) for sh in range(2) for tt in range(NT)]
                for tt, sh in it:
                    probs[tt][sh] = mypool.tile(
                        [128, 2, 512], BF16, tag="probs", name=f"probs_{hp}_{tt}_{sh}"
                    )
                def scores_exp(tt, sh):
                    ssl = slice(sh * 512, (sh + 1) * 512)
                    ps = pssc.tile([128, 1024], F32, tag="sc", name=f"ps_{hp}_{tt}_{sh}")
                    for par in range(2):
                        po = par * 64
                        nc.tensor.matmul(
                            ps[:, par * 512 : (par + 1) * 512],
                            kt_sb[po : po + 64, hp, tt * 128 : (tt + 1) * 128],
                            qt_sb[po : po + 64, hp, ssl],
                            start=True,
                            stop=True,
                        )
                    nc.scalar.activation(
                        out=probs[tt][sh][:],
                        in_=ps[:].rearrange("p (a b) -> p a b", a=2),
                        func=mybir.ActivationFunctionType.Exp,
                        scale=0.125,
                    )

                def normalize(par, sh, po_ps):
                    # po_ps rows 0:64 = unnormalized out^T, rows 64:128 = den
                    # replicated across partitions -> pure DVE chain
                    po = par * 64
                    ssl = slice(sh * 512, (sh + 1) * 512)
                    den = small.tile([64, 512], F32, tag="den", name=f"den_{hp}_{par}_{sh}")
                    nc.vector.tensor_copy(out=den[:], in_=po_ps[A : 2 * A, :])
                    recip = small.tile([64, 512], F32, tag="recip", name=f"rc_{hp}_{par}_{sh}")
                    nc.vector.reciprocal_approx_fast(out=recip[:], in_=den[:])
                    nc.vector.tensor_mul(
                        catt_sb[po : po + 64, hp, ssl], po_ps[0:A, :], recip[:]
                    )

                def pv_mm(po_ps, tt, h, par, sh):
                    # rows 0:64 <- v^T @ expT; rows 64:128 <- den replicated
                    nc.tensor.matmul(
                        po_ps[:],
                        v_sb[:, tt, h, :],
                        probs[tt][sh][:, par, :],
                        start=(tt == 0),
                        stop=(tt == NT - 1),
                    )

                def pv_group(sh, par):
                    po_ps = pspv.tile(
                        [128, 512], F32, tag="pv", name=f"pv_{2 * hp + par}_{sh}"
                    )
                    for t2 in range(NT):
                        pv_mm(po_ps, t2, 2 * hp + par, par, sh)
                    normalize(par, sh, po_ps)

                if hp < NT - 1:
                    # scores tiles run sh-major so PV(sh0) unlocks after 8
                    # exps instead of 16; PV(sh0) and the next pair's
                    # projections then fill the PE while the sh1 exps stream.
                    # Pair period ~ max(PE work, 16 exps + PV(sh1) tail)
                    for tt in range(NT):
                        scores_exp(tt, 0)
                        if tt in (2, 4, 6):
                            qk_group(hp + 1, (tt - 2) // 2)
                        if tt == 7 and hp > 0:
                            z_pass(hp - 1)
                    for tt in range(NT):
                        scores_exp(tt, 1)
                        if tt == 0:
                            qk_group(hp + 1, 3)
                        elif tt == 2:
                            pv_group(0, 0)
                        elif tt == 5:
                            pv_group(0, 1)
                    for par in range(2):
                        pv_group(1, par)
                else:
                    # last pair: interleave PV with scores/exp per t-tile so
                    # the PE keeps work during the exp-paced pipeline drain;
                    # the sh0 half of z + the first four output transposes
                    # ride inside / right after the sh1 stream so only the
                    # sh1 half remains in the tail
                    trig = [nc.sync, nc.scalar, nc.gpsimd]

                    def emit_out(st):
                        # transpose z^T[:, st-tile] -> z [s, 64] and DMA out;
                        # psum from psqk (pspv still holds the live PV tiles)
                        pt = psqk.tile([128, 512], F32, tag="qk", name=f"pt_{st}")
                        nc.tensor.transpose(
                            pt[:, 0:A], zt_sb[:, st * 128 : (st + 1) * 128], ident[:]
                        )
                        nc.vector.tensor_copy(out=out_sb[:, st, :], in_=pt[:, 0:A])
                        trig[st % 3].dma_start(
                            out=out.rearrange("(st p) n -> p st n", p=128)[:, st, :],
                            in_=out_sb[:, st, :],
                        )

                    for sh in range(2):
                        pvt = [
                            pspv.tile([128, 512], F32, tag="pv", name=f"pvL_{par}_{sh}")
                            for par in range(2)
                        ]
                        for tt in range(NT):
                            scores_exp(tt, sh)
                            if sh == 0 and tt == 6:
                                z_pass(hp - 1)
                            if sh == 1 and tt == 6:
                                z_half(NT - 1, 0)
                            for par in range(2):
                                pv_mm(pvt[par], tt, 2 * hp + par, par, sh)
                        if sh == 1:
                            # zt_sb[:, 0:512] is final: move its transposes
                            # off the tail while the sh1 normalize drains
                            for st in range(4):
                                emit_out(st)
                            # hold the HAM clock gate open across the ~4us
                            # normalize-chain wait so the final z matmul and
                            # transposes run warm instead of at 1.2 GHz
                            keep_warm(8)
                        for par in range(2):
                            normalize(par, sh, pvt[par])
            # final s-half of z; probs2 teardown overlaps the tail
            z_half(NT - 1, 1)
            p2stack.close()
            for st in range(4, NT):
                emit_out(st)

    nc.compile()
    return nc


def _get_program():
    global _PROGRAM
    if _PROGRAM is None:
        _PROGRAM = _build_program()
    return _PROGRAM


def kernel(x: np.ndarray, W: np.ndarray, Wz: np.ndarray) -> np.ndarray:
    global LAST_EXEC_NS
    x = np.asarray(x, dtype=np.float32)
    W = np.asarray(W, dtype=np.float32)
    Wz = np.asarray(Wz, dtype=np.float32)
    assert x.shape == (B, S, D) and W.shape == (H, 3, D, A) and Wz.shape == (H * A, A)

    # host-side prep: flatten weights head-major [d, h*a], cast to bf16
    Wf = W.astype(BF)
    wq_h = np.ascontiguousarray(Wf[:, 0].transpose(1, 0, 2).reshape(D, H * A))
    wk_h = np.ascontiguousarray(Wf[:, 1].transpose(1, 0, 2).reshape(D, H * A))
    wv_h = np.ascontiguousarray(Wf[:, 2].transpose(1, 0, 2).reshape(D, H * A))
    wz_h = np.ascontiguousarray(Wz.astype(BF))

    in_maps = []
    for b in range(B):
        xt = np.ascontiguousarray(x[b].T.astype(BF))
        in_maps.append({"xT": xt, "wq": wq_h, "wk": wk_h, "wv": wv_h, "wz": wz_h})

    nc = _get_program()
    last_exc = None
    for attempt in range(3):
        try:
            res = run_bass_kernel_spmd(nc, in_maps, core_ids=list(range(B)), trace=TRACE)
            break
        except Exception as e:  # transient device faults (e.g. NRT unrecoverable)
            last_exc = e
            import time

            time.sleep(2.0)
    else:
        raise last_exc
    LAST_EXEC_NS = res.exec_time_ns
    return np.stack([res.results[b]["out"] for b in range(B)], axis=0)


# revision 69
# speedup vs baseline: 1.0285x; 1.0008x over previous
"""Multi-head self-attention Trainium2 kernel (8-core data parallel).

Reference computation (per batch b):
  q/k/v = einsum('sd,hda->hsa', x[b], W[:,i])       i in {0,1,2}
  scores = q @ k^T / sqrt(64); probs = softmax(scores)
  out = probs @ v; cat = concat heads [s, h*a]; z = cat @ Wz

Strategy per core (1 batch per core):
  - host pre-transposes x[b] -> xT [d, s] and flattens W head-major, all bf16
  - qT,kT computed W-stationary: qT[ha, s] tiles (2 heads per 128-partition
    tile); each pair's projection is interleaved into the PREVIOUS pair's
    exp-paced scores stream so the PE never idles waiting on ScalarE
  - v computed x-stationary in natural [t, ha] layout, stored per head as
    [v (64 cols) | ones (64 cols)]
  - scoresT[t, s] = kT.T @ qT per head (K=64); even/odd heads of a pair run in
    PE row-groups 0-1 / 2-3 concurrently (lhsT base partition 0 / 64)
  - exp on ScalarE with scale=1/8, no max subtraction (|scores/8| <~ 5.5)
  - one PV matmul per (head, s-half): lhsT=[v|ones] M=128 gives psum rows
    0:64 = v^T @ expT (unnorm.) and rows 64:128 = softmax denominator
    replicated across 64 partitions (matmul time is N cycles, M is free)
  - normalize is a pure DVE chain: copy den block, reciprocal_approx_fast,
    multiply -> catT[ha, s] bf16 (no cross-partition broadcast needed)
  - z^T accumulated per head pair: transient psum z-matmuls + DVE add into
    zt_sb [64, s]; each pair's z rides inside the next pair's scores stream
  - tail: 8 PE transposes to z [s, 64] fp32, DMA out; the first 4 transposes
    and the big pool teardowns are pulled into pair-7 compute
"""

import sys
from contextlib import ExitStack

sys.path.insert(0, "/opt/trn_rl_repo")

import numpy as np
import ml_dtypes

import concourse.bass as bass
import concourse.bacc as bacc
import concourse.tile as tile
import concourse.mybir as mybir
from concourse.bass_utils import run_bass_kernel_spmd
from concourse.masks import make_identity

F32 = mybir.dt.float32
BF16 = mybir.dt.bfloat16
BF = ml_dtypes.bfloat16

S = 1024  # sequence length
D = 1024  # model dim
H = 16    # heads
A = 64    # attention dim per head
B = 8     # batch (one per core)
NT = 8    # 128-row tiles per 1024 dim

TRACE = False
LAST_EXEC_NS = None

_PROGRAM = None


def _build_program():
    nc = bacc.Bacc("TRN2", target_bir_lowering=False, debug=False)

    xT = nc.dram_tensor("xT", [D, S], BF16, kind="ExternalInput").ap()
    wq = nc.dram_tensor("wq", [D, H * A], BF16, kind="ExternalInput").ap()
    wk = nc.dram_tensor("wk", [D, H * A], BF16, kind="ExternalInput").ap()
    wv = nc.dram_tensor("wv", [D, H * A], BF16, kind="ExternalInput").ap()
    wz = nc.dram_tensor("wz", [H * A, A], BF16, kind="ExternalInput").ap()
    out = nc.dram_tensor("out", [S, A], F32, kind="ExternalOutput").ap()

    with tile.TileContext(nc) as tc:
        with (
            tc.tile_pool(name="persist", bufs=1) as pers,
            tc.tile_pool(name="small", bufs=4) as small,
            tc.tile_pool(name="pssc", bufs=2, space="PSUM") as pssc,
            tc.tile_pool(name="pspv", bufs=2, space="PSUM") as pspv,
            tc.tile_pool(name="psqk", bufs=2, space="PSUM") as psqk,
        ):
            wz_sb = pers.tile([128, NT, A], BF16)
            # per head: [v (64 cols) | ones (64 cols)] so one PV matmul with
            # M=128 yields out^T on psum rows 0:64 and the softmax denominator
            # replicated on rows 64:128 (matmul time is N cycles, M is free)
            v_sb = pers.tile([128, NT, H, 2 * A], BF16)
            qt_sb = pers.tile([128, NT, S], BF16)
            kt_sb = pers.tile([128, NT, S], BF16)
            catt_sb = pers.tile([128, NT, S], BF16)
            ident = pers.tile([64, 64], F32)
            zt_sb = pers.tile([64, S], F32)
            out_sb = pers.tile([128, NT, A], F32)

            qkstack = ExitStack()
            wqkp = qkstack.enter_context(tc.tile_pool(name="wqkp", bufs=1))
            wstack = ExitStack()
            wvp = wstack.enter_context(tc.tile_pool(name="wvp", bufs=1))

            wv_sb = wvp.tile([128, NT, H * A], BF16)
            xt_sb = wqkp.tile([128, NT, S], BF16)
            wq_sb = wqkp.tile([128, NT, H * A], BF16)
            wk_sb = wqkp.tile([128, NT, H * A], BF16)

            # warmup data first so the PE can start ramping immediately
            warm_sb = pers.tile([128, 256], BF16)
            nc.vector.memset(warm_sb[:], 0.0)

            # input DMAs: wv + x first half (v phase) first, then the rest
            for d in range(NT):
                r = slice(d * 128, (d + 1) * 128)
                nc.sync.dma_start(out=wv_sb[:, d, :], in_=wv[r, :])
                nc.sync.dma_start(out=xt_sb[:, d, 0:512], in_=xT[r, 0:512])
            for d in range(NT):
                r = slice(d * 128, (d + 1) * 128)
                nc.sync.dma_start(out=xt_sb[:, d, 512:1024], in_=xT[r, 512:1024])
            for d in range(NT):
                r = slice(d * 128, (d + 1) * 128)
                nc.sync.dma_start(out=wq_sb[:, d, :], in_=wq[r, :])
                nc.sync.dma_start(out=wk_sb[:, d, :], in_=wk[r, :])
                nc.sync.dma_start(out=wz_sb[:, d, :], in_=wz[r, :])

            # ones blocks for the PV denominator rows; split across two idle
            # engines, needed only by the first PV (~45us in)
            nc.vector.memset(v_sb[:, 0:4, :, A : 2 * A], 1.0)
            nc.gpsimd.memset(v_sb[:, 4:8, :, A : 2 * A], 1.0)
            make_identity(nc, ident)

            # warmup burst: dense dummy matmuls at t=0 lift the PE HAM clock
            # gate to 8/8 before the DMA-paced V phase begins
            _wid = [0]

            def keep_warm(n):
                # dummy matmuls with no data deps: the scheduler slots them
                # into PE-idle stretches, keeping the HAM clock gate at 8/8
                _wid[0] += 1
                pw = pssc.tile([128, 1024], F32, tag="sc", name=f"warm_{_wid[0]}")
                for _ in range(n):
                    nc.tensor.matmul(
                        pw[:, 0:256], warm_sb[:, 0:128], warm_sb[:], start=True, stop=True
                    )

            keep_warm(26)

            def qk_group(hq, g):
                # one Q/K projection psum group: g selects (wq/wk, s-half)
                w_sb, dst = ((wq_sb, qt_sb), (wk_sb, kt_sb))[g // 2]
                sh = g % 2
                pq = psqk.tile([128, 512], F32, tag="qk", name=f"pq_{hq}_{g}")
                ssl = slice(sh * 512, (sh + 1) * 512)
                for d in range(NT):
                    nc.tensor.matmul(
                        pq[:],
                        w_sb[:, d, hq * 128 : (hq + 1) * 128],
                        xt_sb[:, d, ssl],
                        start=(d == 0),
                        stop=(d == NT - 1),
                    )
                nc.vector.tensor_copy(out=dst[:, hq, ssl], in_=pq[:])

            # ---- V (natural [t, ha] layout, x-stationary); pair 0's Q/K
            # projection interleaves into the last two V tiles ----
            for tt in range(NT):
                for nh in range(2):
                    pv = psqk.tile([128, 512], F32, tag="qk")
                    for d in range(NT):
                        nc.tensor.matmul(
                            pv[:],
                            xt_sb[:, d, tt * 128 : (tt + 1) * 128],
                            wv_sb[:, d, nh * 512 : (nh + 1) * 512],
                            start=(d == 0),
                            stop=(d == NT - 1),
                        )
                    nc.vector.tensor_copy(
                        out=v_sb[:, tt, nh * 8 : (nh + 1) * 8, 0:A],
                        in_=pv[:].rearrange("p (h a) -> p h a", h=8),
                    )
                    if tt >= 6:
                        qk_group(0, 2 * (tt - 6) + nh)
                if tt < 3:
                    keep_warm(8 - tt)
            wstack.close()  # frees wv_sb
            pstack = ExitStack()
            ppool = pstack.enter_context(tc.tile_pool(name="probs", bufs=23))

            # ---- fused per head-pair: attention with the NEXT pair's Q^T/K^T
            # projection interleaved into the exp-paced scores stream ----
            p2stack = None
            for hp in range(NT):

                def z_half(hz, sh):
                    # zt_sb[:, sh-half] += Wz[hz-chunk]^T @ catT[hz-chunk];
                    # cross-chunk accumulation on the DVE into SBUF so no
                    # PSUM bank is held across pairs
                    ssl = slice(sh * 512, (sh + 1) * 512)
                    pz = psqk.tile([64, 512], F32, tag="qk", name=f"pz_{hz}_{sh}")
                    nc.tensor.matmul(
                        pz[:],
                        wz_sb[:, hz, :],
                        catt_sb[:, hz, ssl],
                        start=True,
                        stop=True,
                    )
                    if hz == 0:
                        nc.vector.tensor_copy(out=zt_sb[:, ssl], in_=pz[:])
                    else:
                        nc.vector.tensor_add(zt_sb[:, ssl], zt_sb[:, ssl], pz[:])

                def z_pass(hz):
                    z_half(hz, 0)
                    z_half(hz, 1)

                if hp == NT - 1:
                    # x/wq/wk are dead (pair 7's projection ran during pair
                    # 6): free their 48KB now and give the last pair its own
                    # probs pool so its exps are not gated on earlier pairs
                    # releasing slots. Closing ppool here also moves its
                    # teardown semaphore traffic off the kernel tail.
                    pstack.close()
                    qkstack.close()
                    p2stack = ExitStack()
                    ppool2 = p2stack.enter_context(tc.tile_pool(name="probs2", bufs=10))
                    mypool = ppool2
                else:
                    mypool = ppool
                # allocation order must match consumption order for the pool
                # ring: sh-major for the reordered pairs, tt-major for the
                # last pair's interleaved drain
                probs = [[None] * 2 for _ in range(NT)]
                if hp < NT - 1:
                    it = [(tt, sh) for sh in range(2) for tt in range(NT)]
                else:
                    it = [(tt, sh) for sh in range(2) for tt in range(NT)]
                for tt, sh in it:
                    probs[tt][sh] = mypool.tile(
                        [128, 2, 512], BF16, tag="probs", name=f"probs_{hp}_{tt}_{sh}"
                    )
                def scores_exp(tt, sh):
                    ssl = slice(sh * 512, (sh + 1) * 512)
                    ps = pssc.tile([128, 1024], F32, tag="sc", name=f"ps_{hp}_{tt}_{sh}")
                    for par in range(2):
                        po = par * 64
                        nc.tensor.matmul(
                            ps[:, par * 512 : (par + 1) * 512],
                            kt_sb[po : po + 64, hp, tt * 128 : (tt + 1) * 128],
                            qt_sb[po : po + 64, hp, ssl],
                            start=True,
                            stop=True,
                        )
                    nc.scalar.activation(
                        out=probs[tt][sh][:],
                        in_=ps[:].rearrange("p (a b) -> p a b", a=2),
                        func=mybir.ActivationFunctionType.Exp,
                        scale=0.125,
                    )

                def normalize(par, sh, po_ps):
                    # po_ps rows 0:64 = unnormalized out^T, rows 64:128 = den
                    # replicated across partitions -> pure DVE chain
                    po = par * 64
                    ssl = slice(sh * 512, (sh + 1) * 512)
                    den = small.tile([64, 512], F32, tag="den", name=f"den_{hp}_{par}_{sh}")
                    nc.vector.tensor_copy(out=den[:], in_=po_ps[A : 2 * A, :])
                    recip = small.tile([64, 512], F32, tag="recip", name=f"rc_{hp}_{par}_{sh}")
                    nc.vector.reciprocal_approx_fast(out=recip[:], in_=den[:])
                    nc.vector.tensor_mul(
                        catt_sb[po : po + 64, hp, ssl], po_ps[0:A, :], recip[:]
                    )

                def pv_mm(po_ps, tt, h, par, sh):
                    # rows 0:64 <- v^T @ expT; rows 64:128 <- den replicated
                    nc.tensor.matmul(
                        po_ps[:],
                        v_sb[:, tt, h, :],
                        probs[tt][sh][:, par, :],
                        start=(tt == 0),
                        stop=(tt == NT - 1),
                    )

                def pv_group(sh, par):
                    po_ps = pspv.tile(
                        [128, 512], F32, tag="pv", name=f"pv_{2 * hp + par}_{sh}"
                    )
                    for t2 in range(NT):
                        pv_mm(po_ps, t2, 2 * hp + par, par, sh)
                    normalize(par, sh, po_ps)

                if hp < NT - 1:
                    # scores tiles run sh-major so PV(sh0) unlocks after 8
                    # exps instead of 16; PV(sh0) and the next pair's
                    # projections then fill the PE while the sh1 exps stream.
                    # Pair period ~ max(PE work, 16 exps + PV(sh1) tail)
                    for tt in range(NT):
                        scores_exp(tt, 0)
                        if tt in (2, 4, 6):
                            qk_group(hp + 1, (tt - 2) // 2)
                        if tt == 7 and hp > 0:
                            z_pass(hp - 1)
                    for tt in range(NT):
                        scores_exp(tt, 1)
                        if tt == 0:
                            qk_group(hp + 1, 3)
                        elif tt == 2:
                            pv_group(0, 0)
                        elif tt == 5:
                            pv_group(0, 1)
                    for par in range(2):
                        pv_group(1, par)
                else:
                    # last pair: interleave PV with scores/exp per t-tile so
                    # the PE keeps work during the exp-paced pipeline drain;
                    # the sh0 half of z + the first four output transposes
                    # ride inside / right after the sh1 stream so only the
                    # sh1 half remains in the tail
                    trig = [nc.sync, nc.scalar, nc.gpsimd]

                    def emit_out(st, scalar_copy=False):
                        # transpose z^T[:, st-tile] -> z [s, 64] and DMA out;
                        # psum from psqk (pspv still holds the live PV tiles).
                        # The tail copies go on the idle ScalarE so they do
                        # not queue behind the normalize chain on the DVE.
                        pt = psqk.tile([128, 512], F32, tag="qk", name=f"pt_{st}")
                        nc.tensor.transpose(
                            pt[:, 0:A], zt_sb[:, st * 128 : (st + 1) * 128], ident[:]
                        )
                        if scalar_copy:
                            nc.scalar.activation(
                                out=out_sb[:, st, :],
                                in_=pt[:, 0:A],
                                func=mybir.ActivationFunctionType.Copy,
                            )
                        else:
                            nc.vector.tensor_copy(out=out_sb[:, st, :], in_=pt[:, 0:A])
                        trig[st % 3].dma_start(
                            out=out.rearrange("(st p) n -> p st n", p=128)[:, st, :],
                            in_=out_sb[:, st, :],
                        )

                    for sh in range(2):
                        pvt = [
                            pspv.tile([128, 512], F32, tag="pv", name=f"pvL_{par}_{sh}")
                            for par in range(2)
                        ]
                        # pvt[1] lags pvt[0] by two tiles so par0's psum stops
                        # ~1us earlier and its normalize chain overlaps the
                        # par1 PV tail instead of serializing after it
                        for tt in range(NT):
                            scores_exp(tt, sh)
                            if sh == 0 and tt == 6:
                                z_pass(hp - 1)
                            if sh == 1 and tt == 6:
                                z_half(NT - 1, 0)
                            pv_mm(pvt[0], tt, 2 * hp, 0, sh)
                            if tt >= 2:
                                pv_mm(pvt[1], tt - 2, 2 * hp + 1, 1, sh)
                        normalize(0, sh, pvt[0])
                        for t2 in (NT - 2, NT - 1):
                            pv_mm(pvt[1], t2, 2 * hp + 1, 1, sh)
                        if sh == 1:
                            # zt_sb[:, 0:512] is final: move its transposes
                            # off the tail while the sh1 normalize drains
                            for st in range(4):
                                emit_out(st)
                            # hold the HAM clock gate open across the
                            # normalize-chain wait so the final z matmul and
                            # transposes run warm instead of at 1.2 GHz
                            keep_warm(8)
                        normalize(1, sh, pvt[1])
            # final s-half of z; probs2 teardown overlaps the tail
            z_half(NT - 1, 1)
            p2stack.close()
            for st in range(4, NT):
                emit_out(st, scalar_copy=True)

    nc.compile()
    return nc


def _get_program():
    global _PROGRAM
    if _PROGRAM is None:
        _PROGRAM = _build_program()
    return _PROGRAM


def kernel(x: np.ndarray, W: np.ndarray, Wz: np.ndarray) -> np.ndarray:
    global LAST_EXEC_NS
    x = np.asarray(x, dtype=np.float32)
    W = np.asarray(W, dtype=np.float32)
    Wz = np.asarray(Wz, dtype=np.float32)
    assert x.shape == (B, S, D) and W.shape == (H, 3, D, A) and Wz.shape == (H * A, A)

    # host-side prep: flatten weights head-major [d, h*a], cast to bf16
    Wf = W.astype(BF)
    wq_h = np.ascontiguousarray(Wf[:, 0].transpose(1, 0, 2).reshape(D, H * A))
    wk_h = np.ascontiguousarray(Wf[:, 1].transpose(1, 0, 2).reshape(D, H * A))
    wv_h = np.ascontiguousarray(Wf[:, 2].transpose(1, 0, 2).reshape(D, H * A))
    wz_h = np.ascontiguousarray(Wz.astype(BF))

    in_maps = []
    for b in range(B):
        xt = np.ascontiguousarray(x[b].T.astype(BF))
        in_maps.append({"xT": xt, "wq": wq_h, "wk": wk_h, "wv": wv_h, "wz": wz_h})

    nc = _get_program()
    last_exc = None
    for attempt in range(3):
        try:
            res = run_bass_kernel_spmd(nc, in_maps, core_ids=list(range(B)), trace=TRACE)
            break
        except Exception as e:  # transient device faults (e.g. NRT unrecoverable)
            last_exc = e
            import time

            time.sleep(2.0)
    else:
        raise last_exc
    LAST_EXEC_NS = res.exec_time_ns
    return np.stack([res.results[b]["out"] for b in range(B)], axis=0)
